# revision 25
# baseline (speedup 1.0000x reference)
"""Trainium2 Bass kernel for nn_Attention_35905926595471.

Channel-attention (XCA-style) block, data-parallel over batch: 8 samples on
8 NeuronCores. FiLM is folded into per-sample qkv weights on the host (bias via
a ones-channel in the contraction). qkv 1x1 conv runs on PE in bf16; the 3x3
depthwise conv is staged into zero-padded 130-stride planes built from
self-contained 34-row quarters (halo rows recomputed with 1-row matmuls, so
quarters have no cross dependencies and schedule freely). q/k conv taps run as
4x tensor-scalar muls on DVE/ACT/Pool with adds on DVE plus compute-capable
DMA (accum_op=add); v conv taps run as diagonal-weight matmuls accumulating in
PSUM. Grams come from DMA-transposed conv outputs; v-ot3 quarters interleave
into the q/k phase to keep PE busy; softmax is batched to limit ACT table
swaps; the attention map folds into the output projection before the final
matmul, staged to bf16 and upcast by the output DMA.
"""
import numpy as np
from contextlib import ExitStack

import concourse.bacc as bacc
import concourse.bass as bass
import concourse.mybir as mybir
from concourse import tile
from concourse.bass_utils import run_bass_kernel_spmd

import bass_rust
F32 = mybir.dt.float32
BF16 = mybir.dt.bfloat16
FP8 = mybir.dt.float8e4
NPFP8 = mybir.dt.np(mybir.dt.float8e4)
PM = mybir.MatmulPerfMode
VP = bass_rust.VecI64Pair
QS = 2.0 ** -6
DS = 2.0 ** -6
Q8ROWS = 36
TAP_PAIRS = [(0, 6), (1, 7), (2, 8), (3, None), (4, None), (5, None)]
NPBF16 = mybir.dt.np(BF16)
AL = mybir.AluOpType
AX = mybir.AxisListType
AF = mybir.ActivationFunctionType

DIM, HEADS, H, W = 192, 6, 128, 128
HD = DIM // HEADS          # 32
N = H * W                  # 16384
CH = 2048                  # px per chunk (16 rows)
SW = W + 2                 # padded row stride 130
QROWS = 34                 # quarter plane rows: 32 image rows + 2 halo
PROWS = 18                 # ot4 pair plane rows: 16 image rows + 2 halo


def _perm():
    perm = []
    for t in range(3):
        for h in (2 * t, 2 * t + 1):
            perm += list(range(h * HD, (h + 1) * HD))
            perm += list(range(DIM + h * HD, DIM + (h + 1) * HD))
    perm += list(range(2 * DIM, 3 * DIM))
    return np.array(perm)


def _emit(nc, t):
    with ExitStack() as ctx:
        tc = ctx.enter_context(tile.TileContext(nc))
        sb = ctx.enter_context(tc.tile_pool(name="sb", bufs=1))
        plp = ctx.enter_context(tc.tile_pool(name="planes", bufs=3))
        ppp = ctx.enter_context(tc.tile_pool(name="pplanes", bufs=2))
        acp = ctx.enter_context(tc.tile_pool(name="accq", bufs=2))
        xbp = ctx.enter_context(tc.tile_pool(name="xb2", bufs=3))
        xhp = ctx.enter_context(tc.tile_pool(name="xh", bufs=2))
        qtp = ctx.enter_context(tc.tile_pool(name="qkt", bufs=2))
        scr = ctx.enter_context(tc.tile_pool(name="scr", bufs=1))
        ysp = ctx.enter_context(tc.tile_pool(name="ys", bufs=2))
        v4p = ctx.enter_context(tc.tile_pool(name="v4", bufs=2))
        pq = ctx.enter_context(tc.tile_pool(name="pq", bufs=3, space=bass.MemorySpace.PSUM))
        pb = ctx.enter_context(tc.tile_pool(name="pb", bufs=2, space=bass.MemorySpace.PSUM))

        # ---- resident tensors ----
        xa8 = sb.tile([128, 2 * N], FP8, tag="xa8", name="xa8")
        wq8 = sb.tile([128, 1280], FP8, tag="wq8", name="wq8")
        dw8 = sb.tile([128, 4608], FP8, tag="dw8", name="dw8")
        vsb3 = sb.tile([128, N], BF16, tag="vsb3", name="vsb3")
        wq1 = sb.tile([128, 640], BF16, tag="wq1", name="wq1")
        wq2 = sb.tile([65, 640], BF16, tag="wq2", name="wq2")
        wdw = sb.tile([128, 45], F32, tag="wdw", name="wdw")
        dwd = sb.tile([128, 22 * 128], BF16, tag="dwd", name="dwd")
        wpT = [sb.tile([128, DIM], F32, tag="wpT0", name="wpT0"),
               sb.tile([64, DIM], F32, tag="wpT1", name="wpT1")]
        idf = sb.tile([128, 128], F32, tag="idf", name="idf")
        tmpc = sb.tile([128, 3], F32, tag="tmpc", name="tmpc")
        onesr = sb.tile([1, 128], F32, tag="onesr", name="onesr")
        sm = sb.tile([128, 16], F32, tag="sm", name="sm")
        Lsb = [sb.tile([128, 128], F32, tag=f"L{g}", name=f"L{g}") for g in range(3)]
        dscr = sb.tile([128, 128], F32, tag="dscr", name="dscr")
        nrow = [sb.tile([1, 128], F32, tag=f"nrow{g}", name=f"nrow{g}") for g in range(3)]
        Asb = [sb.tile([128, DIM], F32, tag="A0", name="A0"), sb.tile([64, DIM], F32, tag="A1", name="A1")]
        w2t1 = sb.tile([128, DIM], BF16, tag="w2t1", name="w2t1")
        w2t2 = sb.tile([128, DIM], BF16, tag="w2t2", name="w2t2")

        nc.sync.dma_start(wq8[:], t["wq8"].ap()[:, :])
        nc.sync.dma_start(wq1[:], t["wq1"].ap()[:, :])
        nc.sync.dma_start(wq2[:], t["wq2"].ap()[:, :])
        nc.sync.dma_start(wdw[:], t["wdw"].ap()[:, :])
        for ci in range(2):
            nc.sync.dma_start(xa8[:, ci * CH:(ci + 1) * CH], t["xa8"].ap()[:, ci * CH:(ci + 1) * CH])
            nc.sync.dma_start(xa8[:, N + ci * CH:N + (ci + 1) * CH], t["xa8"].ap()[:, N + ci * CH:N + (ci + 1) * CH])
        nc.sync.dma_start(dw8[:], t["dw8"].ap()[:, :])
        nc.sync.dma_start(dwd[:], t["dwd"].ap()[:, :])
        for ci in range(2, 8):
            nc.sync.dma_start(xa8[:, ci * CH:(ci + 1) * CH], t["xa8"].ap()[:, ci * CH:(ci + 1) * CH])
            nc.sync.dma_start(xa8[:, N + ci * CH:N + (ci + 1) * CH], t["xa8"].ap()[:, N + ci * CH:N + (ci + 1) * CH])
        xsv = xa8[:].rearrange("p (j n) -> p j n", j=2)
        wqv = wq8[:].rearrange("p (j o) -> p j o", j=2)
        nc.sync.dma_start(wpT[0][:], t["wpT"].ap()[0:128, :])
        nc.sync.dma_start(wpT[1][:], t["wpT"].ap()[128:192, :])
        nc.sync.dma_start(idf[:], t["idf"].ap()[:, :])
        nc.sync.dma_start(tmpc[:], t["tmpc"].ap()[:, :])
        nc.sync.dma_start(onesr[:], t["onesr"].ap()[:, :])

        def wcol(g, tap):
            return wdw[0:128, g * 9 + tap:g * 9 + tap + 1]

        slab_cache = {}

        def slab_prefetch(c):
            xb = xbp.tile([65, CH], BF16, tag="xb2", name="xb2")
            nc.sync.dma_start(xb[:], t["xb"].ap()[:, c * CH:(c + 1) * CH])
            slab_cache[c] = xb

        def slab(c):
            if c in slab_cache:
                return slab_cache.pop(c)
            xb = xbp.tile([65, CH], BF16, tag="xb2", name="xb2")
            nc.sync.dma_start(xb[:], t["xb"].ap()[:, c * CH:(c + 1) * CH])
            return xb

        slab_prefetch(0)
        slab_prefetch(1)

        def slabA(c):
            xA = xbp.tile([128, CH], BF16, tag="xbA", name="xbA")
            nc.sync.dma_start(xA[:], t["xa"].ap()[:, c * CH:(c + 1) * CH])
            return xA

        def hslabA(px0):
            xh = xhp.tile([128, 128], BF16, tag="xhA", name="xhA")
            nc.sync.dma_start(xh[:], t["xa"].ap()[:, px0:px0 + 128])
            return xh

        def hslab(px0):
            xh = xhp.tile([65, 128], BF16, tag="xh", name="xh")
            nc.sync.dma_start(xh[:], t["xb"].ap()[:, px0:px0 + 128])
            return xh

        def mm_row(ps_region, g, px0, xh, xA, cn=128):
            """1-row (128 px) qkv matmul into a psum region (bf16, v path)."""
            c0 = g * 128
            nc.tensor.matmul(ps_region, wq1[:, c0:c0 + cn], xA[:, 0:128],
                             start=True, stop=False)
            nc.tensor.matmul(ps_region, wq2[0:65, c0:c0 + cn], xh[0:65, 0:128],
                             start=False, stop=True)

        def mm_row8(ps_region, g, px0, cn=128):
            c0 = g * 128
            nc.tensor.matmul(ps_region, wqv[:, :, c0:c0 + cn], xsv[:, :, px0:px0 + 128],
                             start=True, stop=True, perf_mode=PM.DoubleRow)

        # ------- self-contained quarters: fp8/DoubleRow for qk, bf16 for v -------
        def emit_quarter_qk(g, q, taps_fn):
            pl = plp.tile([128, Q8ROWS * SW], FP8, tag="pl8", bufs=2, name="pl8")
            s3 = pl[:].rearrange("p (r c) -> p r c", c=SW)
            nc.gpsimd.memset(s3[:, :, 0:1], 0.0)
            nc.gpsimd.memset(s3[:, :, 129:130], 0.0)
            nc.gpsimd.memset(s3[:, 34:36, :], 0.0)
            if q == 0:
                nc.gpsimd.memset(s3[:, 0:1, :], 0.0)
            if q == 3:
                nc.gpsimd.memset(s3[:, 33:34, :], 0.0)
            hp = pq.tile([128, 512], F32, tag="mm", name="hp")
            h3 = hp[:].rearrange("p (r c) -> p r c", c=W)
            if q > 0:
                mm_row8(hp[:, 0:128], g, (32 * q - 1) * 128)
                nc.scalar.activation(s3[:, 0:1, 1:129], h3[:, 0:1, :], AF.Identity, scale=QS)
            if q < 3:
                mm_row8(hp[:, 128:256], g, (32 * q + 32) * 128)
                nc.scalar.activation(s3[:, 33:34, 1:129], h3[:, 1:2, :], AF.Identity, scale=QS)
            for c in (2 * q, 2 * q + 1):
                lb = 16 * (c % 2)
                for k in range(4):
                    px = c * CH + 512 * k
                    ps = pq.tile([128, 512], F32, tag="mm", name="mm")
                    for nb in range(2):
                        nc.tensor.matmul(ps[:, nb * 256:(nb + 1) * 256],
                                         wqv[:, :, g * 128:(g + 1) * 128],
                                         xsv[:, :, px + nb * 256:px + nb * 256 + 256],
                                         start=True, stop=True, perf_mode=PM.DoubleRow)
                    p3 = ps[:].rearrange("p (r c) -> p r c", c=W)
                    r0 = lb + 1 + 4 * k
                    nc.scalar.activation(s3[:, r0:r0 + 4, 1:129], p3[:], AF.Identity, scale=QS)
            taps_fn(g, 2 * q, pl, 0)
            taps_fn(g, 2 * q + 1, pl, 16)

        def emit_quarter(g, q, taps_fn):
            pl = plp.tile([128, QROWS * SW], BF16, tag="pl", name="pl")
            s3 = pl[:].rearrange("p (r c) -> p r c", c=SW)
            nc.gpsimd.memset(s3[:, :, 0:1], 0.0)
            nc.gpsimd.memset(s3[:, :, 129:130], 0.0)
            if q == 0:
                nc.gpsimd.memset(s3[:, 0:1, :], 0.0)
            if q == 3:
                nc.gpsimd.memset(s3[:, 33:34, :], 0.0)
            hp = pq.tile([128, 512], F32, tag="mm", name="hp")
            h3 = hp[:].rearrange("p (r c) -> p r c", c=W)
            if q > 0:
                px0 = (32 * q - 1) * 128
                mm_row(hp[:, 0:128], g, px0, hslab(px0), hslabA(px0))
                nc.scalar.activation(s3[:, 0:1, 1:129], h3[:, 0:1, :], AF.Identity, scale=1.0)
            if q < 3:
                px0 = (32 * q + 32) * 128
                mm_row(hp[:, 128:256], g, px0, hslab(px0), hslabA(px0))
                nc.scalar.activation(s3[:, 33:34, 1:129], h3[:, 1:2, :], AF.Identity, scale=1.0)
            for c in (2 * q, 2 * q + 1):
                xb = slab(c)
                xA = slabA(c)
                lb = 16 * (c % 2)
                for k in range(4):
                    px = c * CH + 512 * k
                    ps = pq.tile([128, 512], F32, tag="mm", name="mm")
                    nc.tensor.matmul(ps[:], wq1[:, g * 128:(g + 1) * 128],
                                     xA[:, 512 * k:512 * k + 512], start=True, stop=False)
                    nc.tensor.matmul(ps[:], wq2[0:65, g * 128:(g + 1) * 128],
                                     xb[0:65, 512 * k:512 * k + 512], start=False, stop=True)
                    p3 = ps[:].rearrange("p (r c) -> p r c", c=W)
                    r0 = lb + 1 + 4 * k
                    nc.scalar.activation(s3[:, r0:r0 + 4, 1:129], p3[:], AF.Identity, scale=1.0)
            taps_fn(g, 2 * q, pl, 0)
            taps_fn(g, 2 * q + 1, pl, 16)

        # ---- q/k taps: fp8 DoubleRow pair matmuls on PE ----
        def qk_taps(g, c, pl, lb):
            pstride = pl[:].ap[0][0]
            acc = acp.tile([128, CH], BF16, tag="acc", name="acc")
            for kk in range(2):
                tp = pb.tile([128, 1024], F32, tag="vt", name="vt")
                for r in range(8):
                    row = lb + 8 * kk + r
                    for pi, (t0, t1) in enumerate(TAP_PAIRS):
                        dy, dx = t0 // 3, t0 % 3
                        lw = dw8[:, g * 1536 + pi * 256:g * 1536 + pi * 256 + 256]
                        lw = lw.rearrange("p (j m) -> p j m", j=2)
                        rhs = pl[:].copy()
                        rhs.ap = VP([[pstride, 128], [260, 2], [1, 128]])
                        rhs.offset = rhs.offset + (row + dy) * SW + dx
                        nc.tensor.matmul(tp[:, r * 128:(r + 1) * 128], lw, rhs,
                                         start=(pi == 0), stop=(pi == 5),
                                         perf_mode=PM.DoubleRow)
                nc.vector.tensor_scalar_mul(acc[:, kk * 1024:(kk + 1) * 1024], tp[:], DS)
            qt = qtp.tile([128, CH], BF16, tag="qt", name="qt")
            qt3 = qt[:].rearrange("p (b j) -> p b j", j=128)
            nc.sync.dma_start_transpose(qt3, acc[:])
            gram = grams[g]
            for b in range(16):
                nc.tensor.matmul(gram[:], qt[:, b * 128:(b + 1) * 128],
                                 qt[:, b * 128:(b + 1) * 128],
                                 start=(c == 0 and b == 0), stop=(c == 7 and b == 15))

        # ---- v ot3 taps: all-DVE elementwise (PE is the bottleneck) ----
        def v3_taps(g, c, pl, lb):
            s3 = pl[:].rearrange("p (r c) -> p r c", c=SW)

            def view(tap):
                dy, dx = tap // 3, tap % 3
                return s3[:, lb + dy:lb + dy + 16, dx:dx + 128]

            accf = vsb3[0:128, c * CH:(c + 1) * CH]
            a3 = accf.rearrange("p (r c) -> p r c", c=W)
            nc.vector.tensor_scalar_mul(a3, view(4), wcol(3, 4))
            for tap in (0, 1, 2, 3, 5, 6, 7, 8):
                pX = scr.tile([128, CH], BF16, tag="sX", bufs=2, name="sX")
                p3v = pX[:].rearrange("p (r c) -> p r c", c=W)
                nc.vector.tensor_scalar_mul(p3v, view(tap), wcol(3, tap))
                nc.vector.tensor_add(accf, accf, pX[:])

        # ---- v ot4 pair (64 chans packed on partition halves) ----
        v4s = [None] * 4

        def emit_pair(p):
            pp = ppp.tile([128, PROWS * SW], BF16, tag="pp", name="pp")
            s3 = pp[:].rearrange("p (r c) -> p r c", c=SW)
            nc.gpsimd.memset(s3[:, :, 0:1], 0.0)
            nc.gpsimd.memset(s3[:, :, 129:130], 0.0)
            if p == 0:
                nc.gpsimd.memset(s3[0:64, 0:1, :], 0.0)
            if p == 3:
                nc.gpsimd.memset(s3[64:128, 17:18, :], 0.0)
            hp = pq.tile([128, 512], F32, tag="mm", name="hp4")
            h3 = hp[:].rearrange("p (r c) -> p r c", c=W)
            halos = []
            if p > 0:
                halos.append((0, 0, 32 * p - 1, 0))
            halos.append((0, 17, 32 * p + 16, 1))
            halos.append((64, 0, 32 * p + 15, 2))
            if p < 3:
                halos.append((64, 17, 32 * p + 32, 3))
            for (pb0, prow, irow, slot) in halos:
                px0 = irow * 128
                mm_row(hp[pb0:pb0 + 64, slot * 128:(slot + 1) * 128], 4, px0, hslab(px0), hslabA(px0), cn=64)
                nc.scalar.activation(s3[pb0:pb0 + 64, prow:prow + 1, 1:129],
                                     h3[pb0:pb0 + 64, slot:slot + 1, :], AF.Identity, scale=1.0)
            xbe, xbo = slab(2 * p), slab(2 * p + 1)
            xAe, xAo = slabA(2 * p), slabA(2 * p + 1)
            for k in range(4):
                pse = pq.tile([128, 512], F32, tag="mm", name="mm4")
                nc.tensor.matmul(pse[0:64, :], wq1[:, 512:576], xAe[:, 512 * k:512 * k + 512],
                                 start=True, stop=False)
                nc.tensor.matmul(pse[0:64, :], wq2[0:65, 512:576], xbe[0:65, 512 * k:512 * k + 512],
                                 start=False, stop=True)
                nc.tensor.matmul(pse[64:128, :], wq1[:, 512:576], xAo[:, 512 * k:512 * k + 512],
                                 start=True, stop=False)
                nc.tensor.matmul(pse[64:128, :], wq2[0:65, 512:576], xbo[0:65, 512 * k:512 * k + 512],
                                 start=False, stop=True)
                p3 = pse[:].rearrange("p (r c) -> p r c", c=W)
                nc.scalar.activation(s3[:, 1 + 4 * k:5 + 4 * k, 1:129], p3[:], AF.Identity, scale=1.0)
            v4 = v4p.tile([128, CH], BF16, tag="v4", bufs=4, name="v4")
            v4s[p] = v4
            for k in range(2):
                tp = pb.tile([128, 1024], F32, tag="vt", name="vt4")
                t3_ = tp[:].rearrange("p (r c) -> p r c", c=W)
                for q in range(2):
                    for tap in range(9):
                        dy, dx = tap // 3, tap % 3
                        r = 8 * k + 4 * q + dy
                        nc.tensor.matmul(t3_[:, 4 * q:4 * q + 4, :], dwd[:, (9 + tap) * 128:(10 + tap) * 128],
                                         s3[:, r:r + 4, dx:dx + 128], start=(tap == 0), stop=(tap == 8))
                nc.scalar.activation(v4[0:64, k * 1024:k * 1024 + 1024], tp[0:64, :], AF.Identity, scale=1.0)
                nc.scalar.activation(v4[64:128, k * 1024:k * 1024 + 1024], tp[64:128, :], AF.Identity, scale=1.0)

        def norms(g):
            L = Lsb[g]
            dcol = sm[:, 9:10]
            scrc = sm[:, 10:11]
            dsc = sm[:, 11:12]
            nc.vector.tensor_mul(dscr[:], L[:], idf[:])
            nc.vector.reduce_sum(dcol, dscr[:], axis=AX.X)
            nc.scalar.sqrt(scrc, dcol)
            nc.vector.tensor_scalar_max(scrc, scrc, 1e-12)
            nc.vector.reciprocal(dsc, scrc)
            rs = sm[:, 12:13]
            nc.vector.tensor_mul(rs, dsc, tmpc[:, g:g + 1])
            pt = pq.tile([128, 512], F32, tag="mm", name="pt")
            nc.tensor.transpose(pt[0:1, 0:128], dsc, idf[:])
            nc.scalar.copy(nrow[g][:], pt[0:1, 0:128])
            pt2 = pq.tile([128, 512], F32, tag="mm", name="pt2")
            nc.tensor.matmul(pt2[:, 0:128], onesr[:], nrow[g][:], start=True, stop=True)
            nc.vector.tensor_scalar_mul(L[:], L[:], rs)
            nc.vector.tensor_mul(L[:], L[:], pt2[:, 0:128])
            for j in range(2):
                P0, K0 = 64 * j, 64 * j + 32
                mx = sm[P0:P0 + 32, 14:15]
                nc.vector.reduce_max(mx, L[P0:P0 + 32, K0:K0 + 32], axis=AX.X)
                nc.vector.tensor_scalar_sub(L[P0:P0 + 32, K0:K0 + 32], L[P0:P0 + 32, K0:K0 + 32], mx)
                nc.scalar.activation(L[P0:P0 + 32, K0:K0 + 32], L[P0:P0 + 32, K0:K0 + 32], AF.Exp)
                nc.vector.reduce_sum(mx, L[P0:P0 + 32, K0:K0 + 32], axis=AX.X)
                nc.vector.reciprocal(mx, mx)
                nc.vector.tensor_scalar_mul(L[P0:P0 + 32, K0:K0 + 32], L[P0:P0 + 32, K0:K0 + 32], mx)

        # ================= interleaved qk + v phase =================
        grams = {}
        vunits = [("v3", 0), ("p", 0), ("v3", 1), ("p", 1), ("v3", 2), ("p", 2), ("v3", 3), ("p", 3)]
        vit = iter(vunits)
        ILV = {(0, 1), (0, 3), (1, 0), (1, 1), (1, 2), (1, 3), (2, 0), (2, 1)}
        for g in range(3):
            grams[g] = pb.tile([128, 128], F32, tag="gr", bufs=1, name=f"gram{g}")
            for q in range(4):
                emit_quarter_qk(g, q, qk_taps)
                if (g, q) in ILV:
                    kind, idx = next(vit)
                    if kind == "v3":
                        emit_quarter(3, idx, v3_taps)
                    else:
                        emit_pair(idx)
            nc.scalar.copy(Lsb[g][:], grams[g][:])
            norms(g)
        for kind, idx in vit:
            if kind == "v3":
                emit_quarter(3, idx, v3_taps)
            else:
                emit_pair(idx)

        # ================= A_bd + W2T =================
        # ================= A_bd + W2T =================
        nc.gpsimd.memset(Asb[0][:], 0.0)
        nc.gpsimd.memset(Asb[1][:], 0.0)
        for h in range(HEADS):
            g, j = h // 2, h % 2
            src = Lsb[g][64 * j:64 * j + 32, 64 * j + 32:64 * j + 64]
            dst_t = Asb[0] if h < 4 else Asb[1]
            dp = 32 * (h % 4)
            dst = dst_t[dp:dp + 32, 32 * h:32 * h + 32]
            if dp == 64 * j:
                nc.vector.tensor_copy(dst, src)
            else:
                nc.sync.dma_start(dst, src)
        for dt_ in range(2):
            c0, cn = dt_ * 128, (128 if dt_ == 0 else 64)
            ps = pq.tile([128, 512], F32, tag="mm", name="w2")
            nc.tensor.matmul(ps[0:cn, 0:DIM], Asb[0][:, c0:c0 + cn], wpT[0][:], start=True, stop=False)
            nc.tensor.matmul(ps[0:cn, 0:DIM], Asb[1][:, c0:c0 + cn], wpT[1][:], start=False, stop=True)
            if dt_ == 0:
                nc.scalar.copy(w2t1[:], ps[0:128, 0:DIM])
            else:
                nc.scalar.copy(w2t2[0:64, :], ps[0:64, 0:DIM])
        nc.sync.dma_start(w2t2[64:128, :], w2t2[0:64, :])

        # ================= y =================
        def emit_y(ci, v4):
            px = 1024 * ci
            b2 = 64 * ((ci // 2) % 2)
            loc = 1024 * (ci % 2)
            for oT, (o0, on) in enumerate([(0, 128), (128, 64)]):
                yp = pb.tile([128, 1024], F32, tag="vt", name="yp")
                for q in range(2):
                    nc.tensor.matmul(yp[0:on, q * 512:(q + 1) * 512], w2t1[:, o0:o0 + on],
                                     vsb3[0:128, px + q * 512:px + q * 512 + 512], start=True, stop=False)
                    nc.tensor.matmul(yp[0:on, q * 512:(q + 1) * 512], w2t2[b2:b2 + 64, o0:o0 + on],
                                     v4[b2:b2 + 64, loc + q * 512:loc + q * 512 + 512], start=False, stop=True)
                ys = ysp.tile([128, 1024], BF16, tag=("ysA" if oT == 0 else "ysB"), name="ys")
                if oT == 0:
                    nc.scalar.copy(ys[0:on, :], yp[0:on, :])
                else:
                    nc.vector.tensor_copy(ys[0:on, :], yp[0:on, :])
                dst = t["yA"] if oT == 0 else t["yB"]
                nc.sync.dma_start(dst.ap()[:, px:px + 1024], ys[0:on, :])

        for ci in range(16):
            emit_y(ci, v4s[ci // 4])




# revision 27
# speedup vs baseline: 1.0505x; 1.0505x over previous
"""Trainium2 Bass kernel for nn_Attention_35905926595471.

Channel-attention (XCA-style) block, data-parallel over batch: 8 samples on
8 NeuronCores. FiLM is folded into per-sample qkv weights on the host (bias via
a ones-channel in the contraction). qkv 1x1 conv runs on PE in bf16; the 3x3
depthwise conv is staged into zero-padded 130-stride planes built from
self-contained 34-row quarters (halo rows recomputed with 1-row matmuls, so
quarters have no cross dependencies and schedule freely). q/k conv taps run as
4x tensor-scalar muls on DVE/ACT/Pool with adds on DVE plus compute-capable
DMA (accum_op=add); v conv taps run as diagonal-weight matmuls accumulating in
PSUM. Grams come from DMA-transposed conv outputs; v-ot3 quarters interleave
into the q/k phase to keep PE busy; softmax is batched to limit ACT table
swaps; the attention map folds into the output projection before the final
matmul, staged to bf16 and upcast by the output DMA.
"""
import numpy as np
from contextlib import ExitStack

import concourse.bacc as bacc
import concourse.bass as bass
import concourse.mybir as mybir
from concourse import tile
from concourse.bass_utils import run_bass_kernel_spmd

import bass_rust
F32 = mybir.dt.float32
BF16 = mybir.dt.bfloat16
FP8 = mybir.dt.float8e4
NPFP8 = mybir.dt.np(mybir.dt.float8e4)
PM = mybir.MatmulPerfMode
VP = bass_rust.VecI64Pair
QS = 2.0 ** -6
DS = 2.0 ** -6
Q8ROWS = 36
TAP_PAIRS = [(0, 6), (1, 7), (2, 8), (3, None), (4, None), (5, None)]
NPBF16 = mybir.dt.np(BF16)
AL = mybir.AluOpType
AX = mybir.AxisListType
AF = mybir.ActivationFunctionType

DIM, HEADS, H, W = 192, 6, 128, 128
HD = DIM // HEADS          # 32
N = H * W                  # 16384
CH = 2048                  # px per chunk (16 rows)
SW = W + 2                 # padded row stride 130
QROWS = 34                 # quarter plane rows: 32 image rows + 2 halo
PROWS = 18                 # ot4 pair plane rows: 16 image rows + 2 halo


def _perm():
    perm = []
    for t in range(3):
        for h in (2 * t, 2 * t + 1):
            perm += list(range(h * HD, (h + 1) * HD))
            perm += list(range(DIM + h * HD, DIM + (h + 1) * HD))
    perm += list(range(2 * DIM, 3 * DIM))
    return np.array(perm)


def _emit(nc, t):
    with ExitStack() as ctx:
        tc = ctx.enter_context(tile.TileContext(nc))
        sb = ctx.enter_context(tc.tile_pool(name="sb", bufs=1))
        plp = ctx.enter_context(tc.tile_pool(name="planes", bufs=3))
        ppp = ctx.enter_context(tc.tile_pool(name="pplanes", bufs=2))
        acp = ctx.enter_context(tc.tile_pool(name="accq", bufs=2))
        xbp = ctx.enter_context(tc.tile_pool(name="xb2", bufs=3))
        xhp = ctx.enter_context(tc.tile_pool(name="xh", bufs=2))
        qtp = ctx.enter_context(tc.tile_pool(name="qkt", bufs=2))
        scr = ctx.enter_context(tc.tile_pool(name="scr", bufs=1))
        ysp = ctx.enter_context(tc.tile_pool(name="ys", bufs=2))
        v4p = ctx.enter_context(tc.tile_pool(name="v4", bufs=2))
        pq = ctx.enter_context(tc.tile_pool(name="pq", bufs=3, space=bass.MemorySpace.PSUM))
        pb = ctx.enter_context(tc.tile_pool(name="pb", bufs=2, space=bass.MemorySpace.PSUM))

        # ---- resident tensors ----
        xa8 = sb.tile([128, 2 * N], FP8, tag="xa8", name="xa8")
        wq8 = sb.tile([128, 1280], FP8, tag="wq8", name="wq8")
        dw8 = sb.tile([128, 4608], FP8, tag="dw8", name="dw8")
        vsb3 = sb.tile([128, N], BF16, tag="vsb3", name="vsb3")
        wq1 = sb.tile([128, 640], BF16, tag="wq1", name="wq1")
        wq2 = sb.tile([65, 640], BF16, tag="wq2", name="wq2")
        wdw = sb.tile([128, 45], F32, tag="wdw", name="wdw")
        dwd = sb.tile([128, 22 * 128], BF16, tag="dwd", name="dwd")
        wpT = [sb.tile([128, DIM], F32, tag="wpT0", name="wpT0"),
               sb.tile([64, DIM], F32, tag="wpT1", name="wpT1")]
        idf = sb.tile([128, 128], F32, tag="idf", name="idf")
        tmpc = sb.tile([128, 3], F32, tag="tmpc", name="tmpc")
        onesr = sb.tile([1, 128], F32, tag="onesr", name="onesr")
        sm = sb.tile([128, 16], F32, tag="sm", name="sm")
        Lsb = [sb.tile([128, 128], F32, tag=f"L{g}", name=f"L{g}") for g in range(3)]
        dscr = sb.tile([128, 128], F32, tag="dscr", name="dscr")
        nrow = [sb.tile([1, 128], F32, tag=f"nrow{g}", name=f"nrow{g}") for g in range(3)]
        Asb = [sb.tile([128, DIM], F32, tag="A0", name="A0"), sb.tile([64, DIM], F32, tag="A1", name="A1")]
        w2t1 = sb.tile([128, DIM], BF16, tag="w2t1", name="w2t1")
        w2t2 = sb.tile([128, DIM], BF16, tag="w2t2", name="w2t2")

        nc.sync.dma_start(wq8[:], t["wq8"].ap()[:, :])
        nc.sync.dma_start(wq1[:], t["wq1"].ap()[:, :])
        nc.sync.dma_start(wq2[:], t["wq2"].ap()[:, :])
        nc.sync.dma_start(wdw[:], t["wdw"].ap()[:, :])
        for ci in range(2):
            nc.sync.dma_start(xa8[:, ci * CH:(ci + 1) * CH], t["xa8"].ap()[:, ci * CH:(ci + 1) * CH])
            nc.sync.dma_start(xa8[:, N + ci * CH:N + (ci + 1) * CH], t["xa8"].ap()[:, N + ci * CH:N + (ci + 1) * CH])
        nc.sync.dma_start(dw8[:], t["dw8"].ap()[:, :])
        nc.sync.dma_start(dwd[:], t["dwd"].ap()[:, :])
        for ci in range(2, 8):
            nc.sync.dma_start(xa8[:, ci * CH:(ci + 1) * CH], t["xa8"].ap()[:, ci * CH:(ci + 1) * CH])
            nc.sync.dma_start(xa8[:, N + ci * CH:N + (ci + 1) * CH], t["xa8"].ap()[:, N + ci * CH:N + (ci + 1) * CH])
        xsv = xa8[:].rearrange("p (j n) -> p j n", j=2)
        wqv = wq8[:].rearrange("p (j o) -> p j o", j=2)
        nc.sync.dma_start(wpT[0][:], t["wpT"].ap()[0:128, :])
        nc.sync.dma_start(wpT[1][:], t["wpT"].ap()[128:192, :])
        nc.sync.dma_start(idf[:], t["idf"].ap()[:, :])
        nc.sync.dma_start(tmpc[:], t["tmpc"].ap()[:, :])
        nc.sync.dma_start(onesr[:], t["onesr"].ap()[:, :])

        def wcol(g, tap):
            return wdw[0:128, g * 9 + tap:g * 9 + tap + 1]

        slab_cache = {}

        def slab_prefetch(c):
            xb = xbp.tile([65, CH], BF16, tag="xb2", name="xb2")
            nc.sync.dma_start(xb[:], t["xb"].ap()[:, c * CH:(c + 1) * CH])
            slab_cache[c] = xb

        def slab(c):
            if c in slab_cache:
                return slab_cache.pop(c)
            xb = xbp.tile([65, CH], BF16, tag="xb2", name="xb2")
            nc.sync.dma_start(xb[:], t["xb"].ap()[:, c * CH:(c + 1) * CH])
            return xb

        slab_prefetch(0)
        slab_prefetch(1)

        def slabA(c):
            xA = xbp.tile([128, CH], BF16, tag="xbA", name="xbA")
            nc.sync.dma_start(xA[:], t["xa"].ap()[:, c * CH:(c + 1) * CH])
            return xA

        def hslabA(px0):
            xh = xhp.tile([128, 128], BF16, tag="xhA", name="xhA")
            nc.sync.dma_start(xh[:], t["xa"].ap()[:, px0:px0 + 128])
            return xh

        def hslab(px0):
            xh = xhp.tile([65, 128], BF16, tag="xh", name="xh")
            nc.sync.dma_start(xh[:], t["xb"].ap()[:, px0:px0 + 128])
            return xh

        def mm_row(ps_region, g, px0, xh, xA, cn=128):
            """1-row (128 px) qkv matmul into a psum region (bf16, v path)."""
            c0 = g * 128
            nc.tensor.matmul(ps_region, wq1[:, c0:c0 + cn], xA[:, 0:128],
                             start=True, stop=False)
            nc.tensor.matmul(ps_region, wq2[0:65, c0:c0 + cn], xh[0:65, 0:128],
                             start=False, stop=True)

        def mm_row8(ps_region, g, px0, cn=128):
            c0 = g * 128
            nc.tensor.matmul(ps_region, wqv[:, :, c0:c0 + cn], xsv[:, :, px0:px0 + 128],
                             start=True, stop=True, perf_mode=PM.DoubleRow)

        # ------- self-contained quarters: fp8/DoubleRow for qk, bf16 for v -------
        def emit_quarter_qk(g, q, taps_fn):
            pl = plp.tile([128, Q8ROWS * SW], FP8, tag="pl8", name="pl8")
            s3 = pl[:].rearrange("p (r c) -> p r c", c=SW)
            nc.gpsimd.memset(s3[:, :, 0:1], 0.0)
            nc.gpsimd.memset(s3[:, :, 129:130], 0.0)
            nc.gpsimd.memset(s3[:, 34:36, :], 0.0)
            if q == 0:
                nc.gpsimd.memset(s3[:, 0:1, :], 0.0)
            if q == 3:
                nc.gpsimd.memset(s3[:, 33:34, :], 0.0)
            hp = pq.tile([128, 512], F32, tag="mm", name="hp")
            h3 = hp[:].rearrange("p (r c) -> p r c", c=W)
            if q > 0:
                mm_row8(hp[:, 0:128], g, (32 * q - 1) * 128)
                nc.scalar.activation(s3[:, 0:1, 1:129], h3[:, 0:1, :], AF.Identity, scale=QS)
            if q < 3:
                mm_row8(hp[:, 128:256], g, (32 * q + 32) * 128)
                nc.scalar.activation(s3[:, 33:34, 1:129], h3[:, 1:2, :], AF.Identity, scale=QS)
            for c in (2 * q, 2 * q + 1):
                lb = 16 * (c % 2)
                for k in range(4):
                    px = c * CH + 512 * k
                    ps = pq.tile([128, 512], F32, tag="mm", name="mm")
                    for nb in range(2):
                        nc.tensor.matmul(ps[:, nb * 256:(nb + 1) * 256],
                                         wqv[:, :, g * 128:(g + 1) * 128],
                                         xsv[:, :, px + nb * 256:px + nb * 256 + 256],
                                         start=True, stop=True, perf_mode=PM.DoubleRow)
                    p3 = ps[:].rearrange("p (r c) -> p r c", c=W)
                    r0 = lb + 1 + 4 * k
                    nc.scalar.activation(s3[:, r0:r0 + 4, 1:129], p3[:], AF.Identity, scale=QS)
            taps_fn(g, 2 * q, pl, 0)
            taps_fn(g, 2 * q + 1, pl, 16)

        def emit_quarter(g, q, taps_fn):
            pl = plp.tile([128, QROWS * SW], BF16, tag="pl", name="pl")
            s3 = pl[:].rearrange("p (r c) -> p r c", c=SW)
            nc.gpsimd.memset(s3[:, :, 0:1], 0.0)
            nc.gpsimd.memset(s3[:, :, 129:130], 0.0)
            if q == 0:
                nc.gpsimd.memset(s3[:, 0:1, :], 0.0)
            if q == 3:
                nc.gpsimd.memset(s3[:, 33:34, :], 0.0)
            hp = pq.tile([128, 512], F32, tag="mm", name="hp")
            h3 = hp[:].rearrange("p (r c) -> p r c", c=W)
            if q > 0:
                px0 = (32 * q - 1) * 128
                mm_row(hp[:, 0:128], g, px0, hslab(px0), hslabA(px0))
                nc.scalar.activation(s3[:, 0:1, 1:129], h3[:, 0:1, :], AF.Identity, scale=1.0)
            if q < 3:
                px0 = (32 * q + 32) * 128
                mm_row(hp[:, 128:256], g, px0, hslab(px0), hslabA(px0))
                nc.scalar.activation(s3[:, 33:34, 1:129], h3[:, 1:2, :], AF.Identity, scale=1.0)
            for c in (2 * q, 2 * q + 1):
                xb = slab(c)
                xA = slabA(c)
                lb = 16 * (c % 2)
                for k in range(4):
                    px = c * CH + 512 * k
                    ps = pq.tile([128, 512], F32, tag="mm", name="mm")
                    nc.tensor.matmul(ps[:], wq1[:, g * 128:(g + 1) * 128],
                                     xA[:, 512 * k:512 * k + 512], start=True, stop=False)
                    nc.tensor.matmul(ps[:], wq2[0:65, g * 128:(g + 1) * 128],
                                     xb[0:65, 512 * k:512 * k + 512], start=False, stop=True)
                    p3 = ps[:].rearrange("p (r c) -> p r c", c=W)
                    r0 = lb + 1 + 4 * k
                    nc.scalar.activation(s3[:, r0:r0 + 4, 1:129], p3[:], AF.Identity, scale=1.0)
            taps_fn(g, 2 * q, pl, 0)
            taps_fn(g, 2 * q + 1, pl, 16)

        # ---- q/k taps: fp8 DoubleRow pair matmuls on PE ----
        def qk_taps(g, c, pl, lb):
            pstride = pl[:].ap[0][0]
            acc = acp.tile([128, CH], BF16, tag="acc", name="acc")
            for kk in range(2):
                tp = pb.tile([128, 1024], F32, tag="vt", name="vt")
                for r in range(8):
                    row = lb + 8 * kk + r
                    for pi, (t0, t1) in enumerate(TAP_PAIRS):
                        dy, dx = t0 // 3, t0 % 3
                        lw = dw8[:, g * 1536 + pi * 256:g * 1536 + pi * 256 + 256]
                        lw = lw.rearrange("p (j m) -> p j m", j=2)
                        rhs = pl[:].copy()
                        rhs.ap = VP([[pstride, 128], [260, 2], [1, 128]])
                        rhs.offset = rhs.offset + (row + dy) * SW + dx
                        nc.tensor.matmul(tp[:, r * 128:(r + 1) * 128], lw, rhs,
                                         start=(pi == 0), stop=(pi == 5),
                                         perf_mode=PM.DoubleRow)
                nc.vector.tensor_scalar_mul(acc[:, kk * 1024:(kk + 1) * 1024], tp[:], DS)
            qt = qtp.tile([128, CH], BF16, tag="qt", name="qt")
            qt3 = qt[:].rearrange("p (b j) -> p b j", j=128)
            nc.sync.dma_start_transpose(qt3, acc[:])
            gram = grams[g]
            for b in range(16):
                nc.tensor.matmul(gram[:], qt[:, b * 128:(b + 1) * 128],
                                 qt[:, b * 128:(b + 1) * 128],
                                 start=(c == 0 and b == 0), stop=(c == 7 and b == 15))

        # ---- v ot3 taps: all-DVE elementwise (PE is the bottleneck) ----
        def v3_taps(g, c, pl, lb):
            s3 = pl[:].rearrange("p (r c) -> p r c", c=SW)

            def view(tap):
                dy, dx = tap // 3, tap % 3
                return s3[:, lb + dy:lb + dy + 16, dx:dx + 128]

            accf = vsb3[0:128, c * CH:(c + 1) * CH]
            a3 = accf.rearrange("p (r c) -> p r c", c=W)
            nc.vector.tensor_scalar_mul(a3, view(4), wcol(3, 4))
            for tap in (0, 1, 2, 3, 5, 6, 7, 8):
                pX = scr.tile([128, CH], BF16, tag="sX", bufs=1, name="sX")
                p3v = pX[:].rearrange("p (r c) -> p r c", c=W)
                nc.vector.tensor_scalar_mul(p3v, view(tap), wcol(3, tap))
                nc.vector.tensor_add(accf, accf, pX[:])

        # ---- v ot4 pair (64 chans packed on partition halves) ----
        v4s = [None] * 4

        def emit_pair(p):
            pp = ppp.tile([128, PROWS * SW], BF16, tag="pp", name="pp")
            s3 = pp[:].rearrange("p (r c) -> p r c", c=SW)
            nc.gpsimd.memset(s3[:, :, 0:1], 0.0)
            nc.gpsimd.memset(s3[:, :, 129:130], 0.0)
            if p == 0:
                nc.gpsimd.memset(s3[0:64, 0:1, :], 0.0)
            if p == 3:
                nc.gpsimd.memset(s3[64:128, 17:18, :], 0.0)
            hp = pq.tile([128, 512], F32, tag="mm", name="hp4")
            h3 = hp[:].rearrange("p (r c) -> p r c", c=W)
            halos = []
            if p > 0:
                halos.append((0, 0, 32 * p - 1, 0))
            halos.append((0, 17, 32 * p + 16, 1))
            halos.append((64, 0, 32 * p + 15, 2))
            if p < 3:
                halos.append((64, 17, 32 * p + 32, 3))
            for (pb0, prow, irow, slot) in halos:
                px0 = irow * 128
                mm_row(hp[pb0:pb0 + 64, slot * 128:(slot + 1) * 128], 4, px0, hslab(px0), hslabA(px0), cn=64)
                nc.scalar.activation(s3[pb0:pb0 + 64, prow:prow + 1, 1:129],
                                     h3[pb0:pb0 + 64, slot:slot + 1, :], AF.Identity, scale=1.0)
            xbe, xbo = slab(2 * p), slab(2 * p + 1)
            xAe, xAo = slabA(2 * p), slabA(2 * p + 1)
            for k in range(4):
                pse = pq.tile([128, 512], F32, tag="mm", name="mm4")
                nc.tensor.matmul(pse[0:64, :], wq1[:, 512:576], xAe[:, 512 * k:512 * k + 512],
                                 start=True, stop=False)
                nc.tensor.matmul(pse[0:64, :], wq2[0:65, 512:576], xbe[0:65, 512 * k:512 * k + 512],
                                 start=False, stop=True)
                nc.tensor.matmul(pse[64:128, :], wq1[:, 512:576], xAo[:, 512 * k:512 * k + 512],
                                 start=True, stop=False)
                nc.tensor.matmul(pse[64:128, :], wq2[0:65, 512:576], xbo[0:65, 512 * k:512 * k + 512],
                                 start=False, stop=True)
                p3 = pse[:].rearrange("p (r c) -> p r c", c=W)
                nc.scalar.activation(s3[:, 1 + 4 * k:5 + 4 * k, 1:129], p3[:], AF.Identity, scale=1.0)
            v4 = v4p.tile([128, CH], BF16, tag="v4", bufs=4, name="v4")
            v4s[p] = v4
            for k in range(2):
                tp = pb.tile([128, 1024], F32, tag="vt", name="vt4")
                t3_ = tp[:].rearrange("p (r c) -> p r c", c=W)
                for q in range(2):
                    for tap in range(9):
                        dy, dx = tap // 3, tap % 3
                        r = 8 * k + 4 * q + dy
                        nc.tensor.matmul(t3_[:, 4 * q:4 * q + 4, :], dwd[:, (9 + tap) * 128:(10 + tap) * 128],
                                         s3[:, r:r + 4, dx:dx + 128], start=(tap == 0), stop=(tap == 8))
                nc.scalar.activation(v4[0:64, k * 1024:k * 1024 + 1024], tp[0:64, :], AF.Identity, scale=1.0)
                nc.scalar.activation(v4[64:128, k * 1024:k * 1024 + 1024], tp[64:128, :], AF.Identity, scale=1.0)

        # ================= interleaved qk + v phase =================
        grams = {}
        vunits = [("v3", 0), ("p", 0), ("v3", 1), ("p", 1), ("v3", 2), ("p", 2), ("v3", 3), ("p", 3)]
        vit = iter(vunits)
        ILV = {(0, 1), (0, 3), (1, 0), (1, 1), (1, 2), (1, 3), (2, 0), (2, 1)}
        for g in range(3):
            grams[g] = pb.tile([128, 128], F32, tag="gr", bufs=1, name=f"gram{g}")
            for q in range(4):
                emit_quarter_qk(g, q, qk_taps)
                if (g, q) in ILV:
                    kind, idx = next(vit)
                    if kind == "v3":
                        emit_quarter(3, idx, v3_taps)
                    else:
                        emit_pair(idx)
            nc.scalar.copy(Lsb[g][:], grams[g][:])
        for kind, idx in vit:
            if kind == "v3":
                emit_quarter(3, idx, v3_taps)
            else:
                emit_pair(idx)

        # ================= norms + softmax (batched) =================
        for g in range(3):
            L = Lsb[g]
            dcol = sm[:, 9:10]
            scrc = sm[:, 10:11]
            dsc = sm[:, 11:12]
            nc.vector.tensor_mul(dscr[:], L[:], idf[:])
            nc.vector.reduce_sum(dcol, dscr[:], axis=AX.X)
            nc.scalar.sqrt(scrc, dcol)
            nc.vector.tensor_scalar_max(scrc, scrc, 1e-12)
            nc.vector.reciprocal(dsc, scrc)
            rs = sm[:, 12:13]
            nc.vector.tensor_mul(rs, dsc, tmpc[:, g:g + 1])
            pt = pq.tile([128, 512], F32, tag="mm", name="pt")
            nc.tensor.transpose(pt[0:1, 0:128], dsc, idf[:])
            nc.scalar.copy(nrow[g][:], pt[0:1, 0:128])
            pt2 = pq.tile([128, 512], F32, tag="mm", name="pt2")
            nc.tensor.matmul(pt2[:, 0:128], onesr[:], nrow[g][:], start=True, stop=True)
            nc.vector.tensor_scalar_mul(L[:], L[:], rs)
            nc.vector.tensor_mul(L[:], L[:], pt2[:, 0:128])
            for j in range(2):
                P0, K0 = 64 * j, 64 * j + 32
                mx = sm[P0:P0 + 32, 14:15]
                nc.vector.reduce_max(mx, L[P0:P0 + 32, K0:K0 + 32], axis=AX.X)
                nc.vector.tensor_scalar_sub(L[P0:P0 + 32, K0:K0 + 32], L[P0:P0 + 32, K0:K0 + 32], mx)
                nc.scalar.activation(L[P0:P0 + 32, K0:K0 + 32], L[P0:P0 + 32, K0:K0 + 32], AF.Exp)
                nc.vector.reduce_sum(mx, L[P0:P0 + 32, K0:K0 + 32], axis=AX.X)
                nc.vector.reciprocal(mx, mx)
                nc.vector.tensor_scalar_mul(L[P0:P0 + 32, K0:K0 + 32], L[P0:P0 + 32, K0:K0 + 32], mx)

        # ================= A_bd + W2T =================
        nc.gpsimd.memset(Asb[0][:], 0.0)
        nc.gpsimd.memset(Asb[1][:], 0.0)
        for h in range(HEADS):
            g, j = h // 2, h % 2
            src = Lsb[g][64 * j:64 * j + 32, 64 * j + 32:64 * j + 64]
            dst_t = Asb[0] if h < 4 else Asb[1]
            dp = 32 * (h % 4)
            dst = dst_t[dp:dp + 32, 32 * h:32 * h + 32]
            if dp == 64 * j:
                nc.vector.tensor_copy(dst, src)
            else:
                nc.sync.dma_start(dst, src)
        for dt_ in range(2):
            c0, cn = dt_ * 128, (128 if dt_ == 0 else 64)
            ps = pq.tile([128, 512], F32, tag="mm", name="w2")
            nc.tensor.matmul(ps[0:cn, 0:DIM], Asb[0][:, c0:c0 + cn], wpT[0][:], start=True, stop=False)
            nc.tensor.matmul(ps[0:cn, 0:DIM], Asb[1][:, c0:c0 + cn], wpT[1][:], start=False, stop=True)
            if dt_ == 0:
                nc.scalar.copy(w2t1[:], ps[0:128, 0:DIM])
            else:
                nc.scalar.copy(w2t2[0:64, :], ps[0:64, 0:DIM])
        nc.sync.dma_start(w2t2[64:128, :], w2t2[0:64, :])

        # ================= y =================
        def emit_y(ci, v4):
            px = 1024 * ci
            b2 = 64 * ((ci // 2) % 2)
            loc = 1024 * (ci % 2)
            for oT, (o0, on) in enumerate([(0, 128), (128, 64)]):
                yp = pb.tile([128, 1024], F32, tag="vt", name="yp")
                for q in range(2):
                    nc.tensor.matmul(yp[0:on, q * 512:(q + 1) * 512], w2t1[:, o0:o0 + on],
                                     vsb3[0:128, px + q * 512:px + q * 512 + 512], start=True, stop=False)
                    nc.tensor.matmul(yp[0:on, q * 512:(q + 1) * 512], w2t2[b2:b2 + 64, o0:o0 + on],
                                     v4[b2:b2 + 64, loc + q * 512:loc + q * 512 + 512], start=False, stop=True)
                ys = ysp.tile([128, 1024], BF16, tag=("ysA" if oT == 0 else "ysB"), name="ys")
                if oT == 0:
                    nc.scalar.copy(ys[0:on, :], yp[0:on, :])
                else:
                    nc.vector.tensor_copy(ys[0:on, :], yp[0:on, :])
                dst = t["yA"] if oT == 0 else t["yB"]
                nc.sync.dma_start(dst.ap()[:, px:px + 1024], ys[0:on, :])

        for ci in range(16):
            emit_y(ci, v4s[ci // 4])


